# revision 6
# baseline (speedup 1.0000x reference)
# NF5 block-quantized linear (AXSLinearV2) on 8 Trainium2 cores.
#
# y = deq(x) @ deq(w).T + bias, where deq is 32-level NormalFloat (NF5)
# fake-quantization over blocks of 64 along in_features with a
# percentile-clipped (99.9%) scale.
#
# Sharding (4x2 grid): core c -> (ro = c//2, co = c%2)
#   - quant shards: x rows [ro*2048 + co*1024, +1024), w rows [co*2048 + ro*512, +512)
#   - pair AllGather {2ro, 2ro+1} assembles qx rows [ro*2048, +2048)
#   - group-4 AllGather {co, co+2, co+4, co+6} assembles qw rows [co*2048, +2048)
#   - matmul produces y^T shard [2048 o, 2048 r]; host transposes and tiles.
#
# The 32-level nearest-level snap is computed without table lookups via an
# erf-based warp: u = Phi(C*nrm) maps the NF5 levels onto a uniform grid,
# a closed-form correction D(z) = E2*(CA*z + CB*z^3 + CC*|z|) (E2 = exp(z^2/2))
# moves the rounding boundaries onto the exact value-space midpoints, and a
# Newton step from the element's own position recovers the level value.

import os
import sys

for _p in ("/opt/trn_rl_repo", "/root/.axon_site/_ro/trn_rl_repo"):
    if os.path.isdir(_p) and _p not in sys.path:
        sys.path.insert(0, _p)

import numpy as np

C_ = 2.1538746940614564        # ndtri(1 - 1/64)
A_ = 16.0                      # warp: t = A*e + B*|e|  (k-16 domain)
B_ = -0.516129032258065
CA = 0.010089541448848835      # boundary-correction coefficients
CB = -0.00019778768532179383
CC = 0.00028827144861916877
KAPPA = 0.036405892028398036   # D1*sqrt(2*pi)/C
HALFC = 1.0769373470307282     # C/2 (Halley 2nd-order term)
INVC = 0.46427956220347655
ERFS = 0.7071067811865475      # 1/sqrt(2)
QFRAC = 0.9369999999999976     # quantile(0.999) lerp fraction for n=64
MAGIC = 12582912.0             # 1.5*2^23 round-to-nearest trick

NCORES = 8
B_SZ, S_SZ, D_IN, D_OUT = 4, 2048, 4096, 4096
RTOT = B_SZ * S_SZ
XR = RTOT // NCORES            # 1024 x-rows quantized per core
WR = D_OUT // NCORES           # 512 w-rows quantized per core

_cache = {}


def _build_nc():
    import concourse.bass as bass
    import concourse.bacc as bacc
    import concourse.tile as tile
    from concourse import mybir

    f32 = mybir.dt.float32
    bf16 = mybir.dt.bfloat16
    u32 = mybir.dt.uint32
    Alu = mybir.AluOpType
    Act = mybir.ActivationFunctionType

    def bcast64(ap2d):
        """[128, G] AP -> [128, G, 64] AP broadcasting each scalar over 64."""
        return bass.AP(tensor=ap2d.tensor, offset=ap2d.offset,
                       ap=[ap2d.ap[0], ap2d.ap[1], [0, 64]])

    nc = bacc.Bacc("TRN2", target_bir_lowering=False, debug=False,
                   num_devices=NCORES)
    x_sh = nc.dram_tensor("x_sh", [XR, D_IN], f32, kind="ExternalInput")
    w_sh = nc.dram_tensor("w_sh", [WR, D_IN], f32, kind="ExternalInput")
    bias_h = nc.dram_tensor("bias_h", [1, 2048], f32, kind="ExternalInput")
    y_sh = nc.dram_tensor("y_sh", [2048, 2048], f32, kind="ExternalOutput")  # y^T

    with tile.TileContext(nc) as tc:
        with tc.tile_pool(name="dram", bufs=1, space="DRAM") as dram:
            qx_own = dram.tile([XR, D_IN], bf16)
            qx_full = dram.tile([2 * XR, D_IN], bf16)
            qw_own = dram.tile([WR, D_IN], bf16)
            qw_half = dram.tile([4 * WR, D_IN], bf16)

            # ---------------- quantization ----------------
            with (
                tc.tile_pool(name="xin", bufs=2) as xin_pool,
                tc.tile_pool(name="aux", bufs=2) as aux_pool,
                tc.tile_pool(name="scale", bufs=2) as sc_pool,
                tc.tile_pool(name="work", bufs=2) as work,
                tc.tile_pool(name="qout", bufs=2) as qout_pool,
                tc.tile_pool(name="qconst", bufs=1) as qconst,
            ):
                biasCA = qconst.tile([128, 1], f32)
                nc.vector.memset(biasCA[:], CA)

                def quantize(src, dst, nrows):
                    for ch in range(nrows // 128):
                        r0 = ch * 128
                        xt = xin_pool.tile([128, 64, 64], f32, tag="x")
                        nc.sync.dma_start(
                            xt[:], src[r0:r0 + 128, :].rearrange(
                                "r (g e) -> r g e", e=64))
                        for fh in range(2):
                            g0 = fh * 32
                            xv = xt[:, g0:g0 + 32, :]
                            # |x| (bitwise AND on raw bits) on gpsimd
                            ax = aux_pool.tile([128, 32, 64], f32, tag="ax")
                            nc.vector.tensor_scalar(
                                out=ax[:].bitcast(u32), in0=xv.bitcast(u32),
                                scalar1=0x7FFFFFFF, scalar2=None,
                                op0=Alu.bitwise_and)
                            # per-block top-2 via max8
                            m8 = sc_pool.tile([128, 32, 8], f32, tag="m8")
                            for b in range(32):
                                nc.vector.max(out=m8[:, b, :], in_=ax[:, b, :])
                            dd = sc_pool.tile([128, 32], f32, tag="dd")
                            nc.vector.tensor_tensor(
                                out=dd[:], in0=m8[:, :, 0], in1=m8[:, :, 1],
                                op=Alu.subtract)
                            sc = sc_pool.tile([128, 32], f32, tag="sc")
                            nc.vector.scalar_tensor_tensor(
                                out=sc[:], in0=dd[:], scalar=QFRAC,
                                in1=m8[:, :, 1], op0=Alu.mult, op1=Alu.add)
                            nc.vector.tensor_scalar(
                                out=sc[:], in0=sc[:], scalar1=1e-8,
                                scalar2=None, op0=Alu.max)
                            rcpC = sc_pool.tile([128, 32], f32, tag="rcpC")
                            nc.vector.reciprocal(out=rcpC[:], in_=sc[:])
                            nc.vector.tensor_scalar(
                                out=rcpC[:], in0=rcpC[:], scalar1=C_,
                                scalar2=None, op0=Alu.mult)

                            # ---- warp + snap + value (on [128, 2048]) ----
                            z = work.tile([128, 32, 64], f32, tag="z")
                            nc.vector.tensor_tensor(
                                out=z[:], in0=xv, in1=bcast64(rcpC[:]),
                                op=Alu.mult)
                            zf = z[:].rearrange("p g e -> p (g e)")
                            nc.vector.tensor_scalar(
                                out=zf, in0=zf, scalar1=C_, scalar2=-C_,
                                op0=Alu.min, op1=Alu.max)
                            e = work.tile([128, 2048], f32, tag="T2")
                            nc.scalar.activation(out=e[:], in_=zf,
                                                 func=Act.Erf, scale=ERFS)
                            z2 = work.tile([128, 2048], f32, tag="T3")
                            nc.scalar.activation(out=z2[:], in_=zf,
                                                 func=Act.Square)
                            ab = work.tile([128, 2048], f32, tag="T4")
                            nc.scalar.activation(out=ab[:], in_=e[:],
                                                 func=Act.Abs, scale=B_)
                            tt = work.tile([128, 2048], f32, tag="T5")
                            nc.vector.scalar_tensor_tensor(
                                out=tt[:], in0=e[:], scalar=A_, in1=ab[:],
                                op0=Alu.mult, op1=Alu.subtract)
                            E2 = work.tile([128, 2048], f32, tag="T6")
                            nc.scalar.activation(out=E2[:], in_=z2[:],
                                                 func=Act.Exp, scale=0.5)
                            w1 = work.tile([128, 2048], f32, tag="T7")
                            nc.scalar.activation(out=w1[:], in_=z2[:],
                                                 func=Act.Identity,
                                                 bias=biasCA[:], scale=CB)
                            E2z = work.tile([128, 2048], f32, tag="T2")
                            nc.gpsimd.tensor_tensor(
                                out=E2z[:], in0=E2[:], in1=zf, op=Alu.mult)
                            w2 = work.tile([128, 2048], f32, tag="T4")
                            nc.gpsimd.tensor_tensor(
                                out=w2[:], in0=E2z[:], in1=w1[:], op=Alu.mult)
                            azcc = work.tile([128, 2048], f32, tag="T7")
                            nc.scalar.activation(out=azcc[:], in_=E2z[:],
                                                 func=Act.Abs, scale=CC)
                            tc1 = work.tile([128, 2048], f32, tag="T3")
                            nc.vector.scalar_tensor_tensor(
                                out=tc1[:], in0=w2[:], scalar=-1.0, in1=tt[:],
                                op0=Alu.mult, op1=Alu.add)
                            nc.vector.scalar_tensor_tensor(
                                out=tc1[:], in0=azcc[:], scalar=-1.0,
                                in1=tc1[:], op0=Alu.mult, op1=Alu.add)
                            kk = work.tile([128, 2048], f32, tag="T7")
                            nc.vector.tensor_scalar(
                                out=kk[:], in0=tc1[:], scalar1=MAGIC,
                                scalar2=MAGIC, op0=Alu.add, op1=Alu.subtract)
                            nc.vector.tensor_scalar(
                                out=kk[:], in0=kk[:], scalar1=-16.0,
                                scalar2=15.0, op0=Alu.max, op1=Alu.min)
                            du = work.tile([128, 2048], f32, tag="T3")
                            nc.gpsimd.tensor_tensor(
                                out=du[:], in0=tt[:], in1=kk[:],
                                op=Alu.subtract)
                            sN = work.tile([128, 2048], f32, tag="T2")
                            nc.vector.scalar_tensor_tensor(
                                out=sN[:], in0=du[:], scalar=KAPPA, in1=E2[:],
                                op0=Alu.mult, op1=Alu.mult)
                            v = work.tile([128, 2048], f32, tag="T4")
                            nc.vector.scalar_tensor_tensor(
                                out=v[:], in0=zf, scalar=INVC, in1=sN[:],
                                op0=Alu.mult, op1=Alu.subtract)
                            sN2 = work.tile([128, 2048], f32, tag="T3")
                            nc.scalar.activation(out=sN2[:], in_=sN[:],
                                                 func=Act.Square)
                            h1 = work.tile([128, 2048], f32, tag="T7")
                            nc.gpsimd.tensor_tensor(
                                out=h1[:], in0=sN2[:], in1=zf, op=Alu.mult)
                            nc.vector.scalar_tensor_tensor(
                                out=v[:], in0=h1[:], scalar=HALFC, in1=v[:],
                                op0=Alu.mult, op1=Alu.add)
                            dq = qout_pool.tile([128, 32, 64], bf16, tag="dq")
                            nc.vector.tensor_tensor(
                                out=dq[:],
                                in0=v[:].rearrange("p (g e) -> p g e", e=64),
                                in1=bcast64(sc[:]), op=Alu.mult)
                            nc.sync.dma_start(
                                dst[r0:r0 + 128,
                                    g0 * 64:(g0 + 32) * 64].rearrange(
                                        "r (g e) -> r g e", e=64),
                                dq[:])

                quantize(x_sh, qx_own, XR)
                quantize(w_sh, qw_own, WR)

            nc.gpsimd.collective_compute(
                "AllGather", Alu.bypass,
                replica_groups=[[2 * i, 2 * i + 1] for i in range(4)],
                ins=[qx_own.opt()], outs=[qx_full.opt()])
            nc.gpsimd.collective_compute(
                "AllGather", Alu.bypass,
                replica_groups=[[0, 2, 4, 6], [1, 3, 5, 7]],
                ins=[qw_own.opt()], outs=[qw_half.opt()])

            # ---------------- matmul: y^T[o, r] ----------------
            with (
                tc.tile_pool(name="mmx", bufs=1) as mmx,
                tc.tile_pool(name="mmw", bufs=2) as mmw,
                tc.tile_pool(name="mmy", bufs=2) as mmy,
                tc.tile_pool(name="psum", bufs=2, space="PSUM") as pp,
                tc.tile_pool(name="misc", bufs=1) as misc,
            ):
                xT = mmx.tile([128, 32, 2048], bf16)  # [i_part, k, r]
                for k in range(32):
                    for rc in range(16):
                        nc.sync.dma_start_transpose(
                            xT[:, k, rc * 128:(rc + 1) * 128],
                            qx_full[rc * 128:(rc + 1) * 128,
                                    k * 128:(k + 1) * 128])
                ones = misc.tile([1, 512], bf16)
                nc.vector.memset(ones[:], 1.0)
                brow32 = misc.tile([1, 2048], f32)
                nc.sync.dma_start(brow32[:], bias_h[:, :])
                brow = misc.tile([1, 2048], bf16)
                nc.vector.tensor_copy(brow[:], brow32[:])

                for ot in range(16):
                    wT = mmw.tile([128, 32, 128], bf16, tag="wT")
                    for k in range(32):
                        nc.sync.dma_start_transpose(
                            wT[:, k, :],
                            qw_half[ot * 128:(ot + 1) * 128,
                                    k * 128:(k + 1) * 128])
                    ps = pp.tile([128, 2048], f32, tag="ps")
                    for k in range(32):
                        for rc in range(4):
                            nc.tensor.matmul(
                                ps[:, rc * 512:(rc + 1) * 512],
                                lhsT=wT[:, k, :],
                                rhs=xT[:, k, rc * 512:(rc + 1) * 512],
                                start=(k == 0), stop=False)
                    for rc in range(4):
                        nc.tensor.matmul(
                            ps[:, rc * 512:(rc + 1) * 512],
                            lhsT=brow[:, ot * 128:(ot + 1) * 128],
                            rhs=ones[:],
                            start=False, stop=True)
                    yb = mmy.tile([128, 2048], f32, tag="yb")
                    nc.scalar.copy(yb[:], ps[:])
                    nc.sync.dma_start(y_sh[ot * 128:(ot + 1) * 128, :], yb[:])
    nc.compile()
    return nc


def kernel(input, weight, bias):
    from concourse.bass_utils import run_bass_kernel_spmd

    if "nc" not in _cache:
        _cache["nc"] = _build_nc()
    nc = _cache["nc"]

    x2 = np.ascontiguousarray(
        np.asarray(input, dtype=np.float32).reshape(RTOT, D_IN))
    w = np.asarray(weight, dtype=np.float32)
    b = np.asarray(bias, dtype=np.float32)

    in_maps = []
    for c in range(NCORES):
        ro, co = c // 2, c % 2
        xs = np.ascontiguousarray(x2[ro * 2048 + co * 1024:
                                     ro * 2048 + (co + 1) * 1024])
        ws = np.ascontiguousarray(w[co * 2048 + ro * 512:
                                    co * 2048 + (ro + 1) * 512])
        bh = np.ascontiguousarray(b[co * 2048:(co + 1) * 2048]).reshape(1, 2048)
        in_maps.append({"x_sh": xs, "w_sh": ws, "bias_h": bh})

    trace = bool(int(os.environ.get("KERNEL_TRACE", "0")))
    res = run_bass_kernel_spmd(nc, in_maps, core_ids=list(range(NCORES)),
                               trace=trace)
    _cache["exec_time_ns"] = res.exec_time_ns

    y = np.empty((RTOT, D_OUT), dtype=np.float32)
    for c in range(NCORES):
        ro, co = c // 2, c % 2
        y[ro * 2048:(ro + 1) * 2048, co * 2048:(co + 1) * 2048] = \
            res.results[c]["y_sh"].T
    return y.reshape(B_SZ, S_SZ, D_OUT)


# revision 8
# speedup vs baseline: 1.3905x; 1.3905x over previous
# NF5 block-quantized linear (AXSLinearV2) on 8 Trainium2 cores — v2.
#
# v2 restructure vs v1:
#   - W quantized first; qw AllGather early; wT o-quarter-0 resident during
#     x-quant so the matmul pipelines with quantization.
#   - x pair-AllGather at chunk (128-row) granularity: matmul row-tiles start
#     as soon as their chunk has been exchanged.
#   - r-outer matmul (lhsT = x-panel stationary, 512+512 LDWEIGHTS total),
#     y produced in natural [r, o] orientation.
#   - chunk-level Erf batch (one ACT table-set switch pair per 128-row chunk).
#   - engine balance: DVE / ACT / GPSIMD all carry quant passes.

import os
import sys

for _p in ("/opt/trn_rl_repo", "/root/.axon_site/_ro/trn_rl_repo"):
    if os.path.isdir(_p) and _p not in sys.path:
        sys.path.insert(0, _p)

import numpy as np

C_ = 2.1538746940614564        # ndtri(1 - 1/64)
A_ = 16.0                      # warp: t = A*e + B*|e|  (k-16 domain)
B_ = -0.516129032258065
CA = 0.010089541448848835      # boundary-correction coefficients
CB = -0.00019778768532179383
CC = 0.00028827144861916877
KAPPA = 0.036405892028398036   # D1*sqrt(2*pi)/C
INVC = 0.46427956220347655
HC2 = 2.3195880606018394       # C^2/2 (Halley term, for h1 = sN^2 * (z/C))
ERFS = 0.7071067811865475
QFRAC = 0.9369999999999976     # quantile(0.999) lerp fraction for n=64
MAGIC = 12582912.0             # 1.5*2^23 round-to-nearest trick

NCORES = 8
B_SZ, S_SZ, D_IN, D_OUT = 4, 2048, 4096, 4096
RTOT = B_SZ * S_SZ
XR = RTOT // NCORES            # 1024 x-rows quantized per core
WR = D_OUT // NCORES           # 512 w-rows quantized per core
NXCH = XR // 128               # 8 x chunks
NWCH = WR // 128               # 4 w chunks

_cache = {}


def _build_nc(repeat=1, phase="all"):
    import concourse.bass as bass
    import concourse.bacc as bacc
    import concourse.tile as tile
    from concourse import mybir

    f32 = mybir.dt.float32
    bf16 = mybir.dt.bfloat16
    u32 = mybir.dt.uint32
    Alu = mybir.AluOpType
    Act = mybir.ActivationFunctionType

    def bcast64(ap2d):
        return bass.AP(tensor=ap2d.tensor, offset=ap2d.offset,
                       ap=[ap2d.ap[0], ap2d.ap[1], [0, 64]])

    nc = bacc.Bacc("TRN2", target_bir_lowering=False, debug=False,
                   num_devices=NCORES)
    x_sh = nc.dram_tensor("x_sh", [XR, D_IN], f32, kind="ExternalInput")
    w_sh = nc.dram_tensor("w_sh", [WR, D_IN], f32, kind="ExternalInput")
    bias_h = nc.dram_tensor("bias_h", [1, 2048], f32, kind="ExternalInput")
    y_sh = nc.dram_tensor("y_sh", [2048, 2048], f32, kind="ExternalOutput")

    with tile.TileContext(nc) as tc:
     for _rep in range(repeat):
        with tc.tile_pool(name="dram", bufs=1, space="DRAM") as dram:
            qx_own = dram.tile([XR, D_IN], bf16)
            qx_full = dram.tile([NXCH, 256, D_IN], bf16)   # [chunk][pair-slot*128][i]
            qw_own = dram.tile([WR, D_IN], bf16)
            qw_half = dram.tile([4 * WR, D_IN], bf16)

            with (
                tc.tile_pool(name="mmw1", bufs=1) as mmw1,
                tc.tile_pool(name="mmxp", bufs=2) as mmxp,
                tc.tile_pool(name="mmy", bufs=2) as mmy,
                tc.tile_pool(name="psum1", bufs=2, space="PSUM") as pp1,
                tc.tile_pool(name="misc", bufs=1) as misc,
            ):
                ones = misc.tile([1, 128], bf16)
                brow32 = misc.tile([1, 2048], f32)
                brow = misc.tile([1, 2048], bf16)
                if phase in ("all", "mm"):
                    nc.vector.memset(ones[:], 1.0)
                    nc.sync.dma_start(brow32[:], bias_h[:, :])
                    nc.vector.tensor_copy(brow[:], brow32[:])

                # ------------- quantization (+ chunk AGs + pass-1 mm) -------
                with (
                    tc.tile_pool(name="xin", bufs=2) as xin_pool,
                    tc.tile_pool(name="zpool", bufs=2) as zpool,
                    tc.tile_pool(name="epool", bufs=1) as epool,
                    tc.tile_pool(name="aux", bufs=2) as aux_pool,
                    tc.tile_pool(name="scale", bufs=2) as sc_pool,
                    tc.tile_pool(name="scl", bufs=10) as scl_pool,
                    tc.tile_pool(name="work", bufs=2) as work,
                    tc.tile_pool(name="qout", bufs=2) as qout_pool,
                    tc.tile_pool(name="qconst", bufs=1) as qconst,
                ):
                    biasCA = qconst.tile([128, 1], f32)
                    if phase in ("all", "quant"):
                        nc.vector.memset(biasCA[:], CA)

                    def quant_chunk(src, dst, r0):
                        """Quantize src[r0:r0+128, :] -> dst rows (bf16)."""
                        xt = xin_pool.tile([128, 64, 64], f32, tag="x")
                        nc.sync.dma_start(
                            xt[:], src[r0:r0 + 128, :].rearrange(
                                "r (g e) -> r g e", e=64))
                        zfull = zpool.tile([128, 64, 64], f32, tag="z")
                        scs = []
                        for j in range(8):       # scales + z, 512 cols each
                            xv = xt[:, 8 * j:8 * j + 8, :]
                            ax = aux_pool.tile([128, 8, 64], f32, tag="ax")
                            nc.vector.tensor_scalar(
                                out=ax[:].bitcast(u32), in0=xv.bitcast(u32),
                                scalar1=0x7FFFFFFF, scalar2=None,
                                op0=Alu.bitwise_and)
                            m8 = sc_pool.tile([128, 8, 8], f32, tag="m8")
                            for b in range(8):
                                nc.vector.max(out=m8[:, b, :], in_=ax[:, b, :])
                            dd = sc_pool.tile([128, 8], f32, tag="dd")
                            nc.vector.tensor_tensor(
                                out=dd[:], in0=m8[:, :, 0], in1=m8[:, :, 1],
                                op=Alu.subtract)
                            sc = scl_pool.tile([128, 8], f32, tag="sc")
                            nc.vector.scalar_tensor_tensor(
                                out=sc[:], in0=dd[:], scalar=QFRAC,
                                in1=m8[:, :, 1], op0=Alu.mult, op1=Alu.add)
                            nc.vector.tensor_scalar(
                                out=sc[:], in0=sc[:], scalar1=1e-8,
                                scalar2=None, op0=Alu.max)
                            rcpC = sc_pool.tile([128, 8], f32, tag="rcpC")
                            nc.vector.reciprocal(out=rcpC[:], in_=sc[:])
                            nc.vector.tensor_scalar(
                                out=rcpC[:], in0=rcpC[:], scalar1=C_,
                                scalar2=None, op0=Alu.mult)
                            nc.gpsimd.tensor_tensor(
                                out=zfull[:, 8 * j:8 * j + 8, :], in0=xv,
                                in1=bcast64(rcpC[:]), op=Alu.mult)
                            scs.append(sc)
                        zf2 = zfull[:].rearrange("p g e -> p (g e)")
                        nc.vector.tensor_scalar(
                            out=zf2, in0=zf2, scalar1=C_, scalar2=-C_,
                            op0=Alu.min, op1=Alu.max)
                        efull = epool.tile([128, 4096], f32, tag="e")
                        nc.scalar.activation(out=efull[:], in_=zf2,
                                             func=Act.Erf, scale=ERFS)
                        for j in range(8):       # warp + snap + value, 512 cols
                            sl = slice(512 * j, 512 * (j + 1))
                            zj = zfull[:].rearrange("p g e -> p (g e)")[:, sl]
                            ej = efull[:, sl]
                            z2 = work.tile([128, 512], f32, tag="T3")
                            nc.scalar.activation(out=z2[:], in_=zj,
                                                 func=Act.Square)
                            ab = work.tile([128, 512], f32, tag="T4")
                            nc.scalar.activation(out=ab[:], in_=ej,
                                                 func=Act.Abs, scale=B_)
                            tt = work.tile([128, 512], f32, tag="T5")
                            nc.vector.scalar_tensor_tensor(
                                out=tt[:], in0=ej, scalar=A_, in1=ab[:],
                                op0=Alu.mult, op1=Alu.subtract)
                            E2 = work.tile([128, 512], f32, tag="T6")
                            nc.scalar.activation(out=E2[:], in_=z2[:],
                                                 func=Act.Exp, scale=0.5)
                            w1 = work.tile([128, 512], f32, tag="T7")
                            nc.scalar.activation(out=w1[:], in_=z2[:],
                                                 func=Act.Identity,
                                                 bias=biasCA[:], scale=CB)
                            E2z = work.tile([128, 512], f32, tag="T3")
                            nc.gpsimd.tensor_tensor(
                                out=E2z[:], in0=E2[:], in1=zj, op=Alu.mult)
                            w2 = work.tile([128, 512], f32, tag="T4")
                            nc.vector.tensor_tensor(
                                out=w2[:], in0=E2z[:], in1=w1[:], op=Alu.mult)
                            azcc = work.tile([128, 512], f32, tag="T7")
                            nc.scalar.activation(out=azcc[:], in_=E2z[:],
                                                 func=Act.Abs, scale=CC)
                            tc1 = work.tile([128, 512], f32, tag="T3")
                            nc.vector.scalar_tensor_tensor(
                                out=tc1[:], in0=w2[:], scalar=-1.0, in1=tt[:],
                                op0=Alu.mult, op1=Alu.add)
                            nc.vector.scalar_tensor_tensor(
                                out=tc1[:], in0=azcc[:], scalar=-1.0,
                                in1=tc1[:], op0=Alu.mult, op1=Alu.add)
                            kk = work.tile([128, 512], f32, tag="T7")
                            nc.vector.tensor_scalar(
                                out=kk[:], in0=tc1[:], scalar1=MAGIC,
                                scalar2=MAGIC, op0=Alu.add, op1=Alu.subtract)
                            nc.vector.tensor_scalar(
                                out=kk[:], in0=kk[:], scalar1=-16.0,
                                scalar2=15.0, op0=Alu.max, op1=Alu.min)
                            du = work.tile([128, 512], bf16, tag="T8")
                            nc.gpsimd.tensor_tensor(
                                out=du[:], in0=tt[:], in1=kk[:],
                                op=Alu.subtract)
                            sNb = work.tile([128, 512], bf16, tag="T9")
                            nc.vector.scalar_tensor_tensor(
                                out=sNb[:], in0=du[:], scalar=KAPPA,
                                in1=E2[:], op0=Alu.mult, op1=Alu.mult)
                            zb = work.tile([128, 512], bf16, tag="T10")
                            nc.scalar.activation(out=zb[:], in_=zj,
                                                 func=Act.Copy, scale=INVC)
                            v = work.tile([128, 512], bf16, tag="T11")
                            nc.vector.tensor_tensor(
                                out=v[:], in0=zb[:], in1=sNb[:],
                                op=Alu.subtract)
                            sN2 = work.tile([128, 512], bf16, tag="T7")
                            nc.scalar.activation(out=sN2[:], in_=sNb[:],
                                                 func=Act.Square)
                            h1 = work.tile([128, 512], bf16, tag="T3")
                            nc.gpsimd.tensor_tensor(
                                out=h1[:], in0=sN2[:], in1=zb[:], op=Alu.mult)
                            nc.vector.scalar_tensor_tensor(
                                out=v[:], in0=h1[:], scalar=HC2, in1=v[:],
                                op0=Alu.mult, op1=Alu.add)
                            dq = qout_pool.tile([128, 8, 64], bf16, tag="dq")
                            nc.gpsimd.tensor_tensor(
                                out=dq[:],
                                in0=v[:].rearrange("p (g e) -> p g e", e=64),
                                in1=bcast64(scs[j][:]), op=Alu.mult)
                            nc.sync.dma_start(
                                dst[r0:r0 + 128, sl].rearrange(
                                    "r (g e) -> r g e", e=64), dq[:])

                    def mm_rt(ch, s, wt, o0, width, src):
                        """One row-tile of the matmul against resident wt."""
                        xp = mmxp.tile([128, 32, 128], bf16, tag="xp")
                        for k in range(32):
                            nc.sync.dma_start_transpose(
                                xp[:, k, :],
                                src[s * 128:(s + 1) * 128,
                                    k * 128:(k + 1) * 128])
                        ps = pp1.tile([128, width], f32, tag="ps")
                        for k in range(32):
                            for oc in range(width // 512):
                                nc.tensor.matmul(
                                    ps[:, oc * 512:(oc + 1) * 512],
                                    lhsT=xp[:, k, :],
                                    rhs=wt[:, k, oc * 512:(oc + 1) * 512],
                                    start=(k == 0), stop=False)
                        for oc in range(width // 512):
                            nc.tensor.matmul(
                                ps[:, oc * 512:(oc + 1) * 512], lhsT=ones[:],
                                rhs=brow[:, o0 + oc * 512:o0 + (oc + 1) * 512],
                                start=False, stop=True)
                        yb = mmy.tile([128, width], f32, tag="yb")
                        nc.scalar.copy(yb[:], ps[:])
                        rbase = s * 1024 + ch * 128
                        nc.sync.dma_start(
                            y_sh[rbase:rbase + 128, o0:o0 + width], yb[:])

                    # ---- W first, then its AG, then wT quarter-0 ----
                    if phase in ("all", "quant"):
                        for ch in range(NWCH):
                            quant_chunk(w_sh, qw_own, ch * 128)
                        nc.gpsimd.collective_compute(
                            "AllGather", Alu.bypass,
                            replica_groups=[[0, 2, 4, 6], [1, 3, 5, 7]],
                            ins=[qw_own.opt()], outs=[qw_half.opt()])
                    wt0 = mmw1.tile([128, 32, 512], bf16, tag="wt0")
                    if phase in ("all", "mm"):
                        for k in range(32):
                            for oc in range(4):
                                nc.sync.dma_start_transpose(
                                    wt0[:, k, oc * 128:(oc + 1) * 128],
                                    qw_half[oc * 128:(oc + 1) * 128,
                                            k * 128:(k + 1) * 128])

                    # ---- X chunks: quant -> chunk AG -> pass-1 row tiles ----
                    for ch in range(NXCH):
                        if phase in ("all", "quant"):
                            quant_chunk(x_sh, qx_own, ch * 128)
                            nc.gpsimd.collective_compute(
                                "AllGather", Alu.bypass,
                                replica_groups=[[2 * i, 2 * i + 1]
                                                for i in range(4)],
                                ins=[qx_own[ch * 128:(ch + 1) * 128, :].opt()],
                                outs=[qx_full[ch].opt()])
                        if phase in ("all", "mm"):
                            for s in range(2):
                                mm_rt(ch, s, wt0, 0, 512, qx_full[ch])

                # ---- passes 2-4: o in [512, 2048), bigger residency ----
                if phase in ("all", "mm"):
                    with (
                        tc.tile_pool(name="mmw2", bufs=1) as mmw2,
                        tc.tile_pool(name="psum2", bufs=2, space="PSUM") as pp2,
                    ):
                        wt3 = mmw2.tile([128, 32, 1536], bf16)
                        for k in range(32):
                            for oc in range(12):
                                nc.sync.dma_start_transpose(
                                    wt3[:, k, oc * 128:(oc + 1) * 128],
                                    qw_half[512 + oc * 128:512 + (oc + 1) * 128,
                                            k * 128:(k + 1) * 128])
                        for ch in range(NXCH):
                            for s in range(2):
                                xp = mmxp.tile([128, 32, 128], bf16, tag="xp")
                                for k in range(32):
                                    nc.sync.dma_start_transpose(
                                        xp[:, k, :],
                                        qx_full[ch, s * 128:(s + 1) * 128,
                                                k * 128:(k + 1) * 128])
                                ps = pp2.tile([128, 1536], f32, tag="ps2")
                                for k in range(32):
                                    for oc in range(3):
                                        nc.tensor.matmul(
                                            ps[:, oc * 512:(oc + 1) * 512],
                                            lhsT=xp[:, k, :],
                                            rhs=wt3[:, k,
                                                    oc * 512:(oc + 1) * 512],
                                            start=(k == 0), stop=False)
                                for oc in range(3):
                                    nc.tensor.matmul(
                                        ps[:, oc * 512:(oc + 1) * 512],
                                        lhsT=ones[:],
                                        rhs=brow[:, 512 + oc * 512:
                                                 512 + (oc + 1) * 512],
                                        start=False, stop=True)
                                yb = mmy.tile([128, 1536], f32, tag="yb2")
                                nc.scalar.copy(yb[:], ps[:])
                                rbase = s * 1024 + ch * 128
                                nc.sync.dma_start(
                                    y_sh[rbase:rbase + 128, 512:2048], yb[:])
    nc.compile()
    return nc


def kernel(input, weight, bias):
    from concourse.bass_utils import run_bass_kernel_spmd

    if "nc" not in _cache:
        _cache["nc"] = _build_nc(
            repeat=int(os.environ.get("KERNEL_REPEAT", "1")),
            phase=os.environ.get("KERNEL_PHASE", "all"))
    nc = _cache["nc"]

    x2 = np.ascontiguousarray(
        np.asarray(input, dtype=np.float32).reshape(RTOT, D_IN))
    w = np.asarray(weight, dtype=np.float32)
    b = np.asarray(bias, dtype=np.float32)

    in_maps = []
    for c in range(NCORES):
        ro, co = c // 2, c % 2
        xs = np.ascontiguousarray(x2[ro * 2048 + co * 1024:
                                     ro * 2048 + (co + 1) * 1024])
        ws = np.ascontiguousarray(w[co * 2048 + ro * 512:
                                    co * 2048 + (ro + 1) * 512])
        bh = np.ascontiguousarray(b[co * 2048:(co + 1) * 2048]).reshape(1, 2048)
        in_maps.append({"x_sh": xs, "w_sh": ws, "bias_h": bh})

    res = run_bass_kernel_spmd(nc, in_maps, core_ids=list(range(NCORES)))
    _cache["exec_time_ns"] = res.exec_time_ns

    y = np.empty((RTOT, D_OUT), dtype=np.float32)
    for c in range(NCORES):
        ro, co = c // 2, c % 2
        y[ro * 2048:(ro + 1) * 2048, co * 2048:(co + 1) * 2048] = \
            res.results[c]["y_sh"]
    return y.reshape(B_SZ, S_SZ, D_OUT)
